# revision 7
# baseline (speedup 1.0000x reference)
"""Multi-head cross-attention Trainium2 kernel (8 NeuronCores).

Sharding: core c handles batch b = c // 4 and the 4 heads h0 = (c % 4) * 4
.. h0 + 4 (feature block of 256 columns of the QKV projections).
Each core computes its attn slice [4, 1024, 2048] and a partial output
projection [1024, 1024]; the host sums partials over the 4 cores of each
batch and stacks attn slices.

Shapes (fixed): B=2, Lq=1024, Lk=2048, D=1024, H=16, dh=64.
"""

import numpy as np

import concourse.bass as bass
import concourse.tile as tile
from concourse import bacc, mybir
from concourse.bass_utils import run_bass_kernel_spmd
from concourse.masks import make_identity

FP = mybir.dt.float32
P = 128
D = 1024
LQ = 1024
LK = 2048
H_PER = 4          # heads per core
DH = 64
FC = H_PER * DH    # 256 features per core
NDC = D // P       # 8 contraction chunks
NQT = LQ // P      # 8 q tiles
NKC = LK // P      # 16 k chunks
N_CORES = 8

Exp = mybir.ActivationFunctionType.Exp
Identity = mybir.ActivationFunctionType.Identity


def _body(nc, tc, tens, pools):
    qT, kT, vT, wqT, wkT, wvT, woT, bq, bk, bv, bias, biasT, attn, outp = tens
    consts, acts, wpool, proj, biasp, ep, eTp, smallp, ps_score, ps_av, ps_misc = pools

    ident = consts.tile([P, P], FP, tag="ident", name="ident")
    make_identity(nc, ident[:])
    ones_row = consts.tile([1, P], FP, tag="ones_row", name="ones_row")
    nc.gpsimd.memset(ones_row[:], 1.0)
    one11 = consts.tile([1, 1], FP, tag="one11", name="one11")
    nc.gpsimd.memset(one11[:], 1.0)

    # per-partition bias columns for the projection epilogues
    bcol = {}
    for name, src in (("bq", bq), ("bk", bk), ("bv", bv)):
        for fc in range(2):
            t = smallp.tile([P, 1], FP, tag=f"{name}{fc}", bufs=1, name=f"{name}{fc}")
            nc.sync.dma_start(t[:], src[fc * P:(fc + 1) * P, :])
            bcol[(name, fc)] = t
    bv_row = smallp.tile([1, FC], FP, tag="bv_row", bufs=1, name="bv_row")
    nc.sync.dma_start(bv_row[:], bv.rearrange("f one -> one f"))

    # resident projection outputs
    QTt = [proj.tile([P, LQ], FP, tag=f"QT{fc}", name=f"QT{fc}") for fc in range(2)]
    KTt = [proj.tile([P, LK], FP, tag=f"KT{fc}", name=f"KT{fc}") for fc in range(2)]
    Vt = [proj.tile([P, H_PER * (DH + 1)], FP, tag=f"V{kc}", name=f"V{kc}") for kc in range(NKC)]
    OTt = [proj.tile([P, LQ], FP, tag=f"OT{fc}", name=f"OT{fc}") for fc in range(2)]

    # ---------------- stage A: projections ----------------
    # Q: QT[fc] = (wqT[:, fc].T @ qT) + bq  -> [128, 1024] feature-major
    wq = []
    for dc in range(NDC):
        w = wpool.tile([P, FC], FP, tag="w", bufs=8, name="wtile")
        nc.sync.dma_start(w[:], wqT[dc * P:(dc + 1) * P, :])
        wq.append(w)
    psq = [ps_score.tile([P, LQ], FP, tag="score", name="pscore") for _ in range(2)]
    for dc in range(NDC):
        a = acts.tile([P, LQ], FP, tag="act", bufs=8, name="atile")
        nc.sync.dma_start(a[:], qT[dc * P:(dc + 1) * P, :])
        for fc in range(2):
            for qh in range(2):
                nc.tensor.matmul(
                    psq[fc][:, qh * 512:(qh + 1) * 512],
                    wq[dc][:, fc * P:(fc + 1) * P],
                    a[:, qh * 512:(qh + 1) * 512],
                    start=(dc == 0), stop=(dc == NDC - 1),
                )
    for fc in range(2):
        nc.scalar.activation(QTt[fc][:], psq[fc][:], Identity, bias=bcol[("bq", fc)][:])

    # K: KT[fc] = (wkT[:, fc].T @ kT) + bk -> [128, 2048]
    wk = []
    for dc in range(NDC):
        w = wpool.tile([P, FC], FP, tag="w", bufs=8, name="wtile")
        nc.sync.dma_start(w[:], wkT[dc * P:(dc + 1) * P, :])
        wk.append(w)
    for kh in range(2):
        psk = [ps_score.tile([P, LQ], FP, tag="score", name="pscore") for _ in range(2)]
        for dc in range(NDC):
            a = acts.tile([P, LQ], FP, tag="act", bufs=8, name="atile")
            nc.sync.dma_start(a[:], kT[dc * P:(dc + 1) * P, kh * 1024:(kh + 1) * 1024])
            for fc in range(2):
                for sh in range(2):
                    nc.tensor.matmul(
                        psk[fc][:, sh * 512:(sh + 1) * 512],
                        wk[dc][:, fc * P:(fc + 1) * P],
                        a[:, sh * 512:(sh + 1) * 512],
                        start=(dc == 0), stop=(dc == NDC - 1),
                    )
        for fc in range(2):
            nc.scalar.activation(
                KTt[fc][:, kh * 1024:(kh + 1) * 1024], psk[fc][:],
                Identity, bias=bcol[("bk", fc)][:],
            )

    # V: token-major with per-head ones column: V[kc] = [v_h0|1|v_h1|1|...]
    wv = []
    for dc in range(NDC):
        w = wpool.tile([P, FC], FP, tag="w", bufs=8, name="wtile")
        nc.sync.dma_start(w[:], wvT[dc * P:(dc + 1) * P, :])
        wv.append(w)
    for kh in range(2):
        va = []
        for dc in range(NDC):
            a = acts.tile([P, LQ], FP, tag="act", bufs=8, name="atile")
            nc.sync.dma_start(a[:], vT[dc * P:(dc + 1) * P, kh * 1024:(kh + 1) * 1024])
            va.append(a)
        for kcl in range(NKC // 2):
            kc = kh * (NKC // 2) + kcl
            psv = ps_score.tile([P, LQ], FP, tag="score", name="pscore")
            for dc in range(NDC):
                nc.tensor.matmul(
                    psv[:, 0:FC],
                    va[dc][:, kcl * P:(kcl + 1) * P],
                    wv[dc][:],
                    start=(dc == 0), stop=False,
                )
            nc.tensor.matmul(psv[:, 0:FC], ones_row[:], bv_row[:],
                             start=False, stop=True)
            for h in range(H_PER):
                nc.vector.tensor_copy(
                    Vt[kc][:, h * (DH + 1):h * (DH + 1) + DH],
                    psv[:, h * DH:(h + 1) * DH],
                )
                nc.gpsimd.memset(Vt[kc][:, h * (DH + 1) + DH:(h + 1) * (DH + 1)], 1.0)

    # ---------------- stage B: attention per head ----------------
    for h in range(H_PER):
        fch = h // 2
        r0 = (h % 2) * DH
        # transposed side: sT[k, q] = K_h Q_h^T + bias^T ; eT = exp(sT)
        # AV: psum_o[0:64] = sum_k V_h[k, :] eT[k, q]; row 64 = sums[q]
        ps_o = ps_av.tile([P, LQ], FP, tag="av", name="psav")
        for kc in range(NKC):
            ps_t = ps_score.tile([P, LQ], FP, tag="score", name="pscore")
            for sh in range(2):
                nc.tensor.matmul(
                    ps_t[:, sh * 512:(sh + 1) * 512],
                    KTt[fch][r0:r0 + DH, kc * P:(kc + 1) * P],
                    QTt[fch][r0:r0 + DH, sh * 512:(sh + 1) * 512],
                    start=True, stop=False,
                )
            btT = biasp.tile([P, LQ], FP, tag="biasT", bufs=4, name="btT")
            nc.sync.dma_start(btT[:], biasT[h, kc * P:(kc + 1) * P, :])
            for sh in range(2):
                nc.tensor.matmul(
                    ps_t[:, sh * 512:(sh + 1) * 512],
                    ident[:],
                    btT[:, sh * 512:(sh + 1) * 512],
                    start=False, stop=True,
                )
            eTt = eTp.tile([P, LQ], FP, tag="eT", bufs=3, name="eTt")
            nc.scalar.activation(eTt[:], ps_t[:], Exp)
            for sh in range(2):
                nc.tensor.matmul(
                    ps_o[0:DH + 1, sh * 512:(sh + 1) * 512],
                    Vt[kc][:, h * (DH + 1):(h + 1) * (DH + 1)],
                    eTt[:, sh * 512:(sh + 1) * 512],
                    start=(kc == 0), stop=(kc == NKC - 1),
                )

        # r = 1 / sums  (row 64 of ps_o)
        r_sb = smallp.tile([1, LQ], FP, tag="r_sb", bufs=2, name="r_sb")
        nc.vector.reciprocal(r_sb[:], ps_o[DH:DH + 1, :])
        rb = smallp.tile([P, LQ], FP, tag="rb", bufs=2, name="rb")
        nc.gpsimd.partition_broadcast(rb[:], r_sb[:])
        # O^T rows for this head, normalized
        nc.vector.tensor_mul(OTt[fch][r0:r0 + DH, :], ps_o[0:DH, :], rb[0:DH, :])
        # r as per-partition columns for the natural side
        rp = ps_misc.tile([P, NQT], FP, tag="misc", name="rp")
        for qt in range(NQT):
            nc.tensor.matmul(rp[:, qt:qt + 1], r_sb[0:1, qt * P:(qt + 1) * P],
                             one11[:], start=True, stop=True)
        r_col = smallp.tile([P, NQT], FP, tag="r_col", bufs=2, name="r_col")
        nc.vector.tensor_copy(r_col[:], rp[:])

        # natural side: s[q, k] = Q_h K_h^T + bias ; attn = exp(s) * r[q]
        for qt in range(NQT):
            btile = biasp.tile([P, LK], FP, tag="bias", bufs=4, name="btile")
            nc.sync.dma_start(btile[:], bias[h, qt * P:(qt + 1) * P, :])
            for kh in range(2):
                ps_s = ps_score.tile([P, LQ], FP, tag="score", name="pscore")
                for sh in range(2):
                    nc.tensor.matmul(
                        ps_s[:, sh * 512:(sh + 1) * 512],
                        QTt[fch][r0:r0 + DH, qt * P:(qt + 1) * P],
                        KTt[fch][r0:r0 + DH, kh * 1024 + sh * 512:kh * 1024 + (sh + 1) * 512],
                        start=True, stop=False,
                    )
                for sh in range(2):
                    nc.tensor.matmul(
                        ps_s[:, sh * 512:(sh + 1) * 512],
                        ident[:],
                        btile[:, kh * 1024 + sh * 512:kh * 1024 + (sh + 1) * 512],
                        start=False, stop=True,
                    )
                e_t = ep.tile([P, LQ], FP, tag="e", bufs=3, name="e_t")
                nc.scalar.activation(e_t[:], ps_s[:], Exp)
                nc.vector.tensor_scalar_mul(e_t[:], e_t[:], r_col[:, qt:qt + 1])
                nc.scalar.dma_start(
                    attn[h, qt * P:(qt + 1) * P, kh * 1024:(kh + 1) * 1024], e_t[:])

    # ---------------- out projection ----------------
    wo = []
    for fc in range(2):
        w = acts.tile([P, D], FP, tag="act", bufs=8, name="wotile")
        nc.sync.dma_start(w[:], woT[fc * P:(fc + 1) * P, :])
        wo.append(w)
    for qc in range(NQT):
        ps = ps_score.tile([P, D], FP, tag="score", name="psout")
        for fc in range(2):
            for sh in range(2):
                nc.tensor.matmul(
                    ps[:, sh * 512:(sh + 1) * 512],
                    OTt[fc][:, qc * P:(qc + 1) * P],
                    wo[fc][:, sh * 512:(sh + 1) * 512],
                    start=(fc == 0), stop=(fc == 1),
                )
        ot = ep.tile([P, D], FP, tag="e", bufs=3, name="ot")
        nc.scalar.copy(ot[:], ps[:])
        nc.scalar.dma_start(outp[qc * P:(qc + 1) * P, :], ot[:])


def build_nc(reps: int = 1):
    nc = bacc.Bacc("TRN2", target_bir_lowering=False, debug=False,
                   num_devices=N_CORES)
    qT = nc.dram_tensor("qT", [D, LQ], FP, kind="ExternalInput")
    kT = nc.dram_tensor("kT", [D, LK], FP, kind="ExternalInput")
    vT = nc.dram_tensor("vT", [D, LK], FP, kind="ExternalInput")
    wqT = nc.dram_tensor("wqT", [D, FC], FP, kind="ExternalInput")
    wkT = nc.dram_tensor("wkT", [D, FC], FP, kind="ExternalInput")
    wvT = nc.dram_tensor("wvT", [D, FC], FP, kind="ExternalInput")
    woT = nc.dram_tensor("woT", [FC, D], FP, kind="ExternalInput")
    bq = nc.dram_tensor("bq", [FC, 1], FP, kind="ExternalInput")
    bk = nc.dram_tensor("bk", [FC, 1], FP, kind="ExternalInput")
    bv = nc.dram_tensor("bv", [FC, 1], FP, kind="ExternalInput")
    bias = nc.dram_tensor("bias", [H_PER, LQ, LK], FP, kind="ExternalInput")
    biasT = nc.dram_tensor("biasT", [H_PER, LK, LQ], FP, kind="ExternalInput")
    attn = nc.dram_tensor("attn", [H_PER, LQ, LK], FP, kind="ExternalOutput")
    outp = nc.dram_tensor("outp", [LQ, D], FP, kind="ExternalOutput")
    tens = (qT, kT, vT, wqT, wkT, wvT, woT, bq, bk, bv, bias, biasT, attn, outp)

    with tile.TileContext(nc) as tc:
        with (
            tc.tile_pool(name="consts", bufs=1) as consts,
            tc.tile_pool(name="acts", bufs=8) as acts,
            tc.tile_pool(name="w", bufs=4) as wpool,
            tc.tile_pool(name="proj", bufs=1) as proj,
            tc.tile_pool(name="biasp", bufs=8) as biasp,
            tc.tile_pool(name="ep", bufs=3) as ep,
            tc.tile_pool(name="eTp", bufs=3) as eTp,
            tc.tile_pool(name="smallp", bufs=2) as smallp,
            tc.tile_pool(name="ps_score", bufs=2, space="PSUM") as ps_score,
            tc.tile_pool(name="ps_av", bufs=1, space="PSUM") as ps_av,
            tc.tile_pool(name="ps_misc", bufs=1, space="PSUM") as ps_misc,
        ):
            pools = (consts, acts, wpool, proj, biasp, ep, eTp, smallp,
                     ps_score, ps_av, ps_misc)
            if reps == 1:
                _body(nc, tc, tens, pools)
            else:
                with tc.For_i(0, reps, 1):
                    _body(nc, tc, tens, pools)
    nc.compile()
    return nc


def prep_core_inputs(query, key, value, mask, attn_bias, Wq, bq, Wk, bk, Wv, bv, Wo):
    """Build the 8 per-core input dicts (host-side shard + layout prep)."""
    f32 = np.float32
    query = np.asarray(query, f32)
    key = np.asarray(key, f32)
    value = np.asarray(value, f32)
    mask = np.asarray(mask)
    attn_bias = np.asarray(attn_bias, f32)
    Wq = np.asarray(Wq, f32); Wk = np.asarray(Wk, f32)
    Wv = np.asarray(Wv, f32); Wo = np.asarray(Wo, f32)
    bq = np.asarray(bq, f32); bk = np.asarray(bk, f32); bv = np.asarray(bv, f32)

    scale = f32(1.0 / 8.0)  # 1/sqrt(dh)
    per_batch = {}
    for b in range(2):
        per_batch[b] = dict(
            qT=np.ascontiguousarray(query[b].T),
            kT=np.ascontiguousarray(key[b].T),
            vT=np.ascontiguousarray(value[b].T),
            maskneg=np.where(mask[b, 0] == 0, f32(-1e9), f32(0.0)),
        )
    in_maps = []
    for c in range(N_CORES):
        b = c // 4
        h0 = (c % 4) * H_PER
        f0 = h0 * DH
        pb = per_batch[b]
        bias_c = np.ascontiguousarray(
            attn_bias[b, h0:h0 + H_PER] + pb["maskneg"][None, :, :], f32)
        biasT_c = np.ascontiguousarray(bias_c.transpose(0, 2, 1))
        in_maps.append(dict(
            qT=pb["qT"], kT=pb["kT"], vT=pb["vT"],
            wqT=np.ascontiguousarray((scale * Wq[f0:f0 + FC, :]).T),
            wkT=np.ascontiguousarray(Wk[f0:f0 + FC, :].T),
            wvT=np.ascontiguousarray(Wv[f0:f0 + FC, :].T),
            woT=np.ascontiguousarray(Wo[:, f0:f0 + FC].T),
            bq=(scale * bq[f0:f0 + FC]).reshape(FC, 1).astype(f32),
            bk=bk[f0:f0 + FC].reshape(FC, 1).astype(f32),
            bv=bv[f0:f0 + FC].reshape(FC, 1).astype(f32),
            bias=bias_c,
            biasT=biasT_c,
        ))
    return in_maps


def assemble(results, bo):
    attn_full = np.empty((2, 16, LQ, LK), np.float32)
    out_full = np.zeros((2, LQ, D), np.float32)
    for c in range(N_CORES):
        b = c // 4
        h0 = (c % 4) * H_PER
        attn_full[b, h0:h0 + H_PER] = results[c]["attn"]
        out_full[b] += results[c]["outp"]
    out_full += np.asarray(bo, np.float32)[None, None, :]
    return out_full, attn_full


_NC_CACHE = {}


def _get_nc(reps=1):
    if reps not in _NC_CACHE:
        _NC_CACHE[reps] = build_nc(reps)
    return _NC_CACHE[reps]


def kernel(query, key, value, mask, attn_bias, Wq, bq, Wk, bk, Wv, bv, Wo, bo):
    in_maps = prep_core_inputs(query, key, value, mask, attn_bias,
                               Wq, bq, Wk, bk, Wv, bv, Wo)
    nc = _get_nc(1)
    res = run_bass_kernel_spmd(nc, in_maps, core_ids=list(range(N_CORES)))
    return assemble(res.results, bo)
